# revision 2
# baseline (speedup 1.0000x reference)
"""DMTetGeometry kernel for 8 Trainium2 NeuronCores.

Split of work:
  - device (8 cores, data-parallel over vertices): the 5-layer SDF MLP
    (27 -> 256 -> 256 -> 256 -> 256 -> 1), which is all of the FLOPs.
    Activations are kept feature-major ([features, rows]) so every layer
    is a plain K-on-partitions matmul with zero transposes.
  - host: positional encoding (sin/cos must match the fp32 reference
    closely; the ACT engine's table-based Sin with fp32 range reduction
    is not accurate enough for the sign-critical sdf), and the marching
    tetrahedra stage (data-dependent shapes: unique/cumsum/masking).

The output's discrete structure depends on sign(sdf), so the MLP is run
in full fp32 on the PE array.
"""

import sys

for _p in ("/root/.axon_site/_ro/trn_rl_repo", "/opt/trn_rl_repo"):
    if _p not in sys.path:
        sys.path.append(_p)

import numpy as np

import concourse.bacc as bacc
import concourse.mybir as mybir
import concourse.tile as tile
from concourse.bass_utils import run_bass_kernel_spmd

N_CORES = 8
N_VERTS = 200000
PER_CORE = 25000
BLK = 512
NBLK = (PER_CORE + BLK - 1) // BLK  # 49
PER_CORE_PAD = NBLK * BLK  # 25088
D_IN = 27
HID = 256
FREQ_NUM = 4

_TRI_TABLE = np.array([
    [-1,-1,-1,-1,-1,-1],[1,0,2,-1,-1,-1],[4,0,3,-1,-1,-1],[1,4,2,1,3,4],
    [3,1,5,-1,-1,-1],[2,3,0,2,5,3],[1,4,0,1,5,4],[4,2,5,-1,-1,-1],
    [4,5,2,-1,-1,-1],[4,1,0,4,5,1],[3,2,0,3,5,2],[1,3,5,-1,-1,-1],
    [4,1,2,4,3,1],[3,0,4,-1,-1,-1],[2,0,1,-1,-1,-1],[-1,-1,-1,-1,-1,-1]], dtype=np.int32)
_NUM_TRI = np.array([0,1,1,2,1,2,2,1,1,2,2,1,2,1,1,0], dtype=np.int32)
_BASE_EDGES = np.array([0,1,0,2,0,3,1,2,1,3,2,3], dtype=np.int32)

F32 = mybir.dt.float32
Relu = mybir.ActivationFunctionType.Relu
Identity = mybir.ActivationFunctionType.Identity


def _build_nc(nblk=NBLK):
    n_cols = nblk * BLK
    nc = bacc.Bacc("TRN2", target_bir_lowering=False, debug=False,
                   enable_asserts=False)
    x = nc.dram_tensor("x", [D_IN, n_cols], F32, kind="ExternalInput")
    w0 = nc.dram_tensor("w0", [D_IN, HID], F32, kind="ExternalInput")
    # hidden weights pre-chunked on host: [k_chunk, 128, 256]
    wh = [nc.dram_tensor(f"w{l}c", [2, 128, HID], F32, kind="ExternalInput")
          for l in (1, 2, 3)]
    # biases pre-chunked on host: [128, 2]
    bs = [nc.dram_tensor(f"b{l}c", [128, 2], F32, kind="ExternalInput")
          for l in (0, 1, 2, 3)]
    wf = nc.dram_tensor("wfc", [128, 2], F32, kind="ExternalInput")
    bf = nc.dram_tensor("bfc", [1, 1], F32, kind="ExternalInput")
    sdf = nc.dram_tensor("sdf", [1, n_cols], F32, kind="ExternalOutput")

    with tile.TileContext(nc) as tc:
        with (
            tc.tile_pool(name="consts", bufs=1) as cpool,
            tc.tile_pool(name="xin", bufs=4) as xpool,
            tc.tile_pool(name="acts", bufs=10) as hpool,
            tc.tile_pool(name="souts", bufs=4) as spool,
            tc.tile_pool(name="ps", bufs=6, space="PSUM") as ppool,
            tc.tile_pool(name="psf", bufs=2, space="PSUM") as pfpool,
        ):
            w0_sb = cpool.tile([D_IN, HID], F32, tag="w0")
            nc.sync.dma_start(out=w0_sb[:], in_=w0[:, :])
            wh_sb = []
            for li, w in enumerate(wh):
                pair = []
                for k in range(2):
                    t = cpool.tile([128, HID], F32, tag=f"w{li}{k}")
                    nc.sync.dma_start(out=t[:], in_=w[k])
                    pair.append(t)
                wh_sb.append(pair)
            bs_sb = []
            for li, b in enumerate(bs):
                t = cpool.tile([128, 2], F32, tag=f"b{li}")
                nc.sync.dma_start(out=t[:], in_=b[:, :])
                bs_sb.append(t)
            wf_sb = cpool.tile([128, 2], F32, tag="wf")
            nc.sync.dma_start(out=wf_sb[:], in_=wf[:, :])
            bf_sb = cpool.tile([1, 1], F32, tag="bf")
            nc.sync.dma_start(out=bf_sb[:], in_=bf[:, :])

            def bias_relu(h, ps, b_ap, use_act):
                if use_act:
                    nc.scalar.activation(h[:], ps[:], Relu, bias=b_ap)
                else:
                    nc.vector.tensor_scalar(
                        h[:], ps[:], b_ap, 0.0,
                        mybir.AluOpType.add, mybir.AluOpType.max)

            for i in range(nblk):
                col = slice(i * BLK, (i + 1) * BLK)
                xt = xpool.tile([D_IN, BLK], F32, tag="xt")
                nc.sync.dma_start(out=xt[:], in_=x[:, col])

                # layer 0: 27 -> 256
                hcur = []
                for m in range(2):
                    ps = ppool.tile([128, BLK], F32, tag="ps")
                    nc.tensor.matmul(ps[:], lhsT=w0_sb[:, m * 128:(m + 1) * 128],
                                     rhs=xt[:], start=True, stop=True)
                    h = hpool.tile([128, BLK], F32, tag="h")
                    bias_relu(h, ps, bs_sb[0][:, m:m + 1], use_act=(m == 0))
                    hcur.append(h)

                # layers 1..3: 256 -> 256
                for li in range(3):
                    hnew = []
                    for m in range(2):
                        ps = ppool.tile([128, BLK], F32, tag="ps")
                        for k in range(2):
                            nc.tensor.matmul(
                                ps[:],
                                lhsT=wh_sb[li][k][:, m * 128:(m + 1) * 128],
                                rhs=hcur[k][:],
                                start=(k == 0), stop=(k == 1))
                        h = hpool.tile([128, BLK], F32, tag="h")
                        bias_relu(h, ps, bs_sb[li + 1][:, m:m + 1],
                                  use_act=(m == 0))
                        hnew.append(h)
                    hcur = hnew

                # final: 256 -> 1
                psf = pfpool.tile([1, BLK], F32, tag="psf")
                for k in range(2):
                    nc.tensor.matmul(psf[:], lhsT=wf_sb[:, k:k + 1],
                                     rhs=hcur[k][:],
                                     start=(k == 0), stop=(k == 1))
                so = spool.tile([1, BLK], F32, tag="so")
                nc.scalar.activation(so[:], psf[:], Identity,
                                     bias=bf_sb[0:1, 0:1])
                nc.sync.dma_start(out=sdf[:, col], in_=so[:])

    nc.compile()
    return nc


_NC_CACHE = {}


def _get_nc():
    if "nc" not in _NC_CACHE:
        _NC_CACHE["nc"] = _build_nc()
    return _NC_CACHE["nc"]


def _posenc_t(pos):
    """Feature-major positional encoding [27, N], fp32, matching the
    reference's fp32 elementwise ops."""
    n = pos.shape[0]
    posT = np.ascontiguousarray(pos.T.astype(np.float32, copy=False))  # [3, N]
    x = np.empty((D_IN, n), np.float32)
    x[0:3] = posT
    for i in range(FREQ_NUM):
        f = np.float32(float(2 ** i) * np.pi)
        fx = f * posT
        x[3 + 6 * i: 6 + 6 * i] = np.sin(fx)
        x[6 + 6 * i: 9 + 6 * i] = np.cos(fx)
    return x


def _chunk_inputs(w0, b0, w1, b1, w2, b2, w3, b3, wf, bf):
    f32 = lambda a: np.ascontiguousarray(np.asarray(a), dtype=np.float32)
    out = {"w0": f32(w0)}
    for name, w in (("w1c", w1), ("w2c", w2), ("w3c", w3)):
        w = f32(w)
        out[name] = np.ascontiguousarray(np.stack([w[:128], w[128:]]))
    for name, b in (("b0c", b0), ("b1c", b1), ("b2c", b2), ("b3c", b3)):
        b = f32(b)
        out[name] = np.ascontiguousarray(b.reshape(2, 128).T)
    wf = f32(wf).reshape(-1)
    out["wfc"] = np.ascontiguousarray(wf.reshape(2, 128).T)
    out["bfc"] = f32(bf).reshape(1, 1)
    return out


def _sdf_device(x_t, weight_maps):
    """x_t: [27, N_VERTS] fp32 -> sdf [N_VERTS] fp32 via 8-core SPMD."""
    nc = _get_nc()
    in_maps = []
    for c in range(N_CORES):
        xc = np.zeros((D_IN, PER_CORE_PAD), np.float32)
        xc[:, :PER_CORE] = x_t[:, c * PER_CORE:(c + 1) * PER_CORE]
        m = {"x": xc}
        m.update(weight_maps)
        in_maps.append(m)
    res = run_bass_kernel_spmd(nc, in_maps, list(range(N_CORES)))
    sdf = np.empty(N_VERTS, np.float32)
    for c in range(N_CORES):
        sdf[c * PER_CORE:(c + 1) * PER_CORE] = \
            res.results[c]["sdf"][0, :PER_CORE]
    return sdf


def _march_tets(pos, tet_fx4, sdf):
    """Marching tetrahedra, numpy mirror of the jnp reference."""
    occ = sdf > 0
    occ_fx4 = occ[tet_fx4]                       # [F,4]
    occ_sum = occ_fx4.sum(-1)
    valid = (occ_sum > 0) & (occ_sum < 4)
    vt = tet_fx4[valid]                          # [V,4]
    occ_v = occ_fx4[valid]

    e = vt[:, _BASE_EDGES].reshape(-1, 2)        # [6V,2]
    lo = np.minimum(e[:, 0], e[:, 1]).astype(np.int64)
    hi = np.maximum(e[:, 0], e[:, 1]).astype(np.int64)
    key = lo * N_VERTS + hi
    uniq_key, inv = np.unique(key, return_inverse=True)
    u_lo = (uniq_key // N_VERTS).astype(np.int64)
    u_hi = (uniq_key % N_VERTS).astype(np.int64)

    cross = (occ[u_lo].astype(np.int32) + occ[u_hi].astype(np.int32)) == 1
    mapping = np.where(cross, np.cumsum(cross, dtype=np.int64) - 1, -1)
    idx_map = mapping[inv].reshape(-1, 6).astype(np.int32)

    ilo = u_lo[cross]
    ihi = u_hi[cross]
    p0 = pos[ilo]
    p1 = pos[ihi]
    s0 = sdf[ilo]
    s1 = sdf[ihi]
    denom = s0 - s1
    verts = (p0 * (-s1)[:, None] + p1 * s0[:, None]) / denom[:, None]

    tetindex = (occ_v.astype(np.int32) *
                np.array([1, 2, 4, 8], np.int32)).sum(-1)
    ntri = _NUM_TRI[tetindex]
    tri = _TRI_TABLE[tetindex]
    m1 = ntri == 1
    m2 = ntri == 2
    f1 = np.take_along_axis(idx_map[m1], tri[m1][:, :3], axis=1).reshape(-1, 3)
    f2 = np.take_along_axis(idx_map[m2], tri[m2][:, :6], axis=1).reshape(-1, 3)
    faces = np.concatenate([f1, f2], axis=0).astype(np.int32)
    return verts.astype(np.float32), faces


def kernel(pos, tet_fx4, w0, b0, w1, b1, w2, b2, w3, b3, wf, bf):
    pos = np.ascontiguousarray(np.asarray(pos), dtype=np.float32)
    tet_fx4 = np.ascontiguousarray(np.asarray(tet_fx4), dtype=np.int32)
    x_t = _posenc_t(pos)
    wm = _chunk_inputs(w0, b0, w1, b1, w2, b2, w3, b3, wf, bf)
    sdf = _sdf_device(x_t, wm)
    return _march_tets(pos, tet_fx4, sdf)


# revision 7
# speedup vs baseline: 1.0408x; 1.0408x over previous
"""DMTetGeometry kernel for 8 Trainium2 NeuronCores.

Split of work:
  - device (8 cores, data-parallel over vertices): the 5-layer SDF MLP
    (27 -> 256 -> 256 -> 256 -> 256 -> 1), which is all of the FLOPs.
    Activations are kept feature-major ([features, rows]) so every layer
    is a plain K-on-partitions matmul with zero transposes.
  - host: positional encoding (sin/cos must match the fp32 reference
    closely; the ACT engine's table-based Sin with fp32 range reduction
    is not accurate enough for the sign-critical sdf), and the marching
    tetrahedra stage (data-dependent shapes: unique/cumsum/masking).

The output's discrete structure depends on sign(sdf), so the MLP is run
in full fp32 on the PE array.
"""

import sys

for _p in ("/root/.axon_site/_ro/trn_rl_repo", "/opt/trn_rl_repo"):
    if _p not in sys.path:
        sys.path.append(_p)

import numpy as np

import concourse.bacc as bacc
import concourse.mybir as mybir
import concourse.tile as tile
from concourse.bass_utils import run_bass_kernel_spmd

N_CORES = 8
N_VERTS = 200000
PER_CORE = 25000
BLK = 512
NBLK = (PER_CORE + BLK - 1) // BLK  # 49
PER_CORE_PAD = NBLK * BLK  # 25088
D_IN = 27
HID = 256
FREQ_NUM = 4

_TRI_TABLE = np.array([
    [-1,-1,-1,-1,-1,-1],[1,0,2,-1,-1,-1],[4,0,3,-1,-1,-1],[1,4,2,1,3,4],
    [3,1,5,-1,-1,-1],[2,3,0,2,5,3],[1,4,0,1,5,4],[4,2,5,-1,-1,-1],
    [4,5,2,-1,-1,-1],[4,1,0,4,5,1],[3,2,0,3,5,2],[1,3,5,-1,-1,-1],
    [4,1,2,4,3,1],[3,0,4,-1,-1,-1],[2,0,1,-1,-1,-1],[-1,-1,-1,-1,-1,-1]], dtype=np.int32)
_NUM_TRI = np.array([0,1,1,2,1,2,2,1,1,2,2,1,2,1,1,0], dtype=np.int32)
_BASE_EDGES = np.array([0,1,0,2,0,3,1,2,1,3,2,3], dtype=np.int32)

F32 = mybir.dt.float32
F32R = mybir.dt.float32r
Relu = mybir.ActivationFunctionType.Relu
Identity = mybir.ActivationFunctionType.Identity
USE_F32R = False


def _mm(ap):
    return ap.bitcast(F32R) if USE_F32R else ap


def _build_nc(nblk=NBLK):
    n_cols = nblk * BLK
    nc = bacc.Bacc("TRN2", target_bir_lowering=False, debug=False,
                   enable_asserts=False)
    x = nc.dram_tensor("x", [D_IN, n_cols], F32, kind="ExternalInput")
    w0 = nc.dram_tensor("w0", [D_IN, HID], F32, kind="ExternalInput")
    # hidden weights pre-chunked on host: [k_chunk, 128, 256]
    wh = [nc.dram_tensor(f"w{l}c", [2, 128, HID], F32, kind="ExternalInput")
          for l in (1, 2, 3)]
    # biases pre-chunked on host: [128, 2]
    bs = [nc.dram_tensor(f"b{l}c", [128, 2], F32, kind="ExternalInput")
          for l in (0, 1, 2, 3)]
    wf = nc.dram_tensor("wfc", [128, 2], F32, kind="ExternalInput")
    bf = nc.dram_tensor("bfc", [1, 1], F32, kind="ExternalInput")
    sdf = nc.dram_tensor("sdf", [1, n_cols], F32, kind="ExternalOutput")

    with tile.TileContext(nc) as tc:
        with (
            tc.tile_pool(name="consts", bufs=1) as cpool,
            tc.tile_pool(name="xin", bufs=4) as xpool,
            tc.tile_pool(name="acts", bufs=10) as hpool,
            tc.tile_pool(name="souts", bufs=4) as spool,
            tc.tile_pool(name="ps", bufs=6, space="PSUM") as ppool,
            tc.tile_pool(name="psf", bufs=2, space="PSUM") as pfpool,
        ):
            w0_sb = cpool.tile([D_IN, HID], F32, tag="w0")
            nc.sync.dma_start(out=w0_sb[:], in_=w0[:, :])
            wh_sb = []
            for li, w in enumerate(wh):
                pair = []
                for k in range(2):
                    t = cpool.tile([128, HID], F32, tag=f"w{li}{k}")
                    nc.sync.dma_start(out=t[:], in_=w[k])
                    pair.append(t)
                wh_sb.append(pair)
            bs_sb = []
            for li, b in enumerate(bs):
                t = cpool.tile([128, 2], F32, tag=f"b{li}")
                nc.sync.dma_start(out=t[:], in_=b[:, :])
                bs_sb.append(t)
            wf_sb = cpool.tile([128, 2], F32, tag="wf")
            nc.sync.dma_start(out=wf_sb[:], in_=wf[:, :])
            bf_sb = cpool.tile([1, 1], F32, tag="bf")
            nc.sync.dma_start(out=bf_sb[:], in_=bf[:, :])

            # HAM warm-up: keep the PE busy during the weight-DMA
            # preamble so the clock gate is at 2.4 GHz when real work
            # arrives (~3.4 us of sustained activity flips it).
            warm = cpool.tile([128, 128], F32, tag="warm")
            nc.gpsimd.memset(warm[:], 0.0)
            wps = pfpool.tile([128, 64], F32, tag="psf")
            for _ in range(34):
                nc.tensor.matmul(wps[:], lhsT=_mm(warm[:]),
                                 rhs=_mm(warm[:, :64]), start=True, stop=True)

            def bias_relu(h, ps, b_ap, use_act):
                if use_act:
                    nc.scalar.activation(h[:], ps[:], Relu, bias=b_ap)
                else:
                    nc.vector.tensor_scalar(
                        h[:], ps[:], b_ap, 0.0,
                        mybir.AluOpType.add, mybir.AluOpType.max)

            # process blocks in pairs so consecutive matmuls share the
            # same stationary (lhsT) operand
            for i0 in range(0, nblk, 2):
                blks = [i0] if i0 + 1 >= nblk else [i0, i0 + 1]
                nb = len(blks)
                cols = [slice(i * BLK, (i + 1) * BLK) for i in blks]
                xts = []
                for b in range(nb):
                    xt = xpool.tile([D_IN, BLK], F32, tag="xt")
                    nc.sync.dma_start(out=xt[:], in_=x[:, cols[b]])
                    xts.append(xt)

                # layer 0: 27 -> 256
                pss = {}
                for m in range(2):
                    for b in range(nb):
                        ps = ppool.tile([128, BLK], F32, tag="ps")
                        nc.tensor.matmul(
                            ps[:], lhsT=_mm(w0_sb[:, m * 128:(m + 1) * 128]),
                            rhs=_mm(xts[b][:]), start=True, stop=True)
                        pss[(b, m)] = ps
                hcur = [[None, None] for _ in range(nb)]
                for m in range(2):
                    for b in range(nb):
                        h = hpool.tile([128, BLK], F32, tag="h")
                        bias_relu(h, pss[(b, m)], bs_sb[0][:, m:m + 1],
                                  use_act=(m == 0))
                        hcur[b][m] = h

                # layers 1..3: 256 -> 256
                for li in range(3):
                    pss = {}
                    for m in range(2):
                        for k in range(2):
                            for b in range(nb):
                                if k == 0:
                                    ps = ppool.tile([128, BLK], F32, tag="ps")
                                    pss[(b, m)] = ps
                                nc.tensor.matmul(
                                    pss[(b, m)][:],
                                    lhsT=_mm(wh_sb[li][k][:, m * 128:(m + 1) * 128]),
                                    rhs=_mm(hcur[b][k][:]),
                                    start=(k == 0), stop=(k == 1))
                    hnew = [[None, None] for _ in range(nb)]
                    for m in range(2):
                        for b in range(nb):
                            h = hpool.tile([128, BLK], F32, tag="h")
                            bias_relu(h, pss[(b, m)], bs_sb[li + 1][:, m:m + 1],
                                      use_act=(m == 0))
                            hnew[b][m] = h
                    hcur = hnew

                # final: 256 -> 1
                psfs = []
                for k in range(2):
                    for b in range(nb):
                        if k == 0:
                            psfs.append(pfpool.tile([1, BLK], F32, tag="psf",
                                                    name="psf"))
                        nc.tensor.matmul(psfs[b][:], lhsT=_mm(wf_sb[:, k:k + 1]),
                                         rhs=_mm(hcur[b][k][:]),
                                         start=(k == 0), stop=(k == 1))
                for b in range(nb):
                    so = spool.tile([1, BLK], F32, tag="so")
                    nc.scalar.activation(so[:], psfs[b][:], Identity,
                                         bias=bf_sb[0:1, 0:1])
                    nc.sync.dma_start(out=sdf[:, cols[b]], in_=so[:])

    nc.compile()
    return nc


_NC_CACHE = {}


def _get_nc():
    if "nc" not in _NC_CACHE:
        _NC_CACHE["nc"] = _build_nc()
    return _NC_CACHE["nc"]


def _posenc_t(pos):
    """Feature-major positional encoding [27, N], fp32, matching the
    reference's fp32 elementwise ops."""
    n = pos.shape[0]
    posT = np.ascontiguousarray(pos.T.astype(np.float32, copy=False))  # [3, N]
    x = np.empty((D_IN, n), np.float32)
    x[0:3] = posT
    for i in range(FREQ_NUM):
        f = np.float32(float(2 ** i) * np.pi)
        fx = f * posT
        x[3 + 6 * i: 6 + 6 * i] = np.sin(fx)
        x[6 + 6 * i: 9 + 6 * i] = np.cos(fx)
    return x


def _chunk_inputs(w0, b0, w1, b1, w2, b2, w3, b3, wf, bf):
    f32 = lambda a: np.ascontiguousarray(np.asarray(a), dtype=np.float32)
    out = {"w0": f32(w0)}
    for name, w in (("w1c", w1), ("w2c", w2), ("w3c", w3)):
        w = f32(w)
        out[name] = np.ascontiguousarray(np.stack([w[:128], w[128:]]))
    for name, b in (("b0c", b0), ("b1c", b1), ("b2c", b2), ("b3c", b3)):
        b = f32(b)
        out[name] = np.ascontiguousarray(b.reshape(2, 128).T)
    wf = f32(wf).reshape(-1)
    out["wfc"] = np.ascontiguousarray(wf.reshape(2, 128).T)
    out["bfc"] = f32(bf).reshape(1, 1)
    return out


def _sdf_device(x_t, weight_maps):
    """x_t: [27, N_VERTS] fp32 -> sdf [N_VERTS] fp32 via 8-core SPMD."""
    nc = _get_nc()
    in_maps = []
    for c in range(N_CORES):
        xc = np.zeros((D_IN, PER_CORE_PAD), np.float32)
        xc[:, :PER_CORE] = x_t[:, c * PER_CORE:(c + 1) * PER_CORE]
        m = {"x": xc}
        m.update(weight_maps)
        in_maps.append(m)
    res = run_bass_kernel_spmd(nc, in_maps, list(range(N_CORES)))
    sdf = np.empty(N_VERTS, np.float32)
    for c in range(N_CORES):
        sdf[c * PER_CORE:(c + 1) * PER_CORE] = \
            res.results[c]["sdf"][0, :PER_CORE]
    return sdf


def _march_tets(pos, tet_fx4, sdf):
    """Marching tetrahedra, numpy mirror of the jnp reference."""
    occ = sdf > 0
    occ_fx4 = occ[tet_fx4]                       # [F,4]
    occ_sum = occ_fx4.sum(-1)
    valid = (occ_sum > 0) & (occ_sum < 4)
    vt = tet_fx4[valid]                          # [V,4]
    occ_v = occ_fx4[valid]

    e = vt[:, _BASE_EDGES].reshape(-1, 2)        # [6V,2]
    lo = np.minimum(e[:, 0], e[:, 1]).astype(np.int64)
    hi = np.maximum(e[:, 0], e[:, 1]).astype(np.int64)
    key = lo * N_VERTS + hi
    uniq_key, inv = np.unique(key, return_inverse=True)
    u_lo = (uniq_key // N_VERTS).astype(np.int64)
    u_hi = (uniq_key % N_VERTS).astype(np.int64)

    cross = (occ[u_lo].astype(np.int32) + occ[u_hi].astype(np.int32)) == 1
    mapping = np.where(cross, np.cumsum(cross, dtype=np.int64) - 1, -1)
    idx_map = mapping[inv].reshape(-1, 6).astype(np.int32)

    ilo = u_lo[cross]
    ihi = u_hi[cross]
    p0 = pos[ilo]
    p1 = pos[ihi]
    s0 = sdf[ilo]
    s1 = sdf[ihi]
    denom = s0 - s1
    verts = (p0 * (-s1)[:, None] + p1 * s0[:, None]) / denom[:, None]

    tetindex = (occ_v.astype(np.int32) *
                np.array([1, 2, 4, 8], np.int32)).sum(-1)
    ntri = _NUM_TRI[tetindex]
    tri = _TRI_TABLE[tetindex]
    m1 = ntri == 1
    m2 = ntri == 2
    f1 = np.take_along_axis(idx_map[m1], tri[m1][:, :3], axis=1).reshape(-1, 3)
    f2 = np.take_along_axis(idx_map[m2], tri[m2][:, :6], axis=1).reshape(-1, 3)
    faces = np.concatenate([f1, f2], axis=0).astype(np.int32)
    return verts.astype(np.float32), faces


def kernel(pos, tet_fx4, w0, b0, w1, b1, w2, b2, w3, b3, wf, bf):
    pos = np.ascontiguousarray(np.asarray(pos), dtype=np.float32)
    tet_fx4 = np.ascontiguousarray(np.asarray(tet_fx4), dtype=np.int32)
    x_t = _posenc_t(pos)
    wm = _chunk_inputs(w0, b0, w1, b1, w2, b2, w3, b3, wf, bf)
    sdf = _sdf_device(x_t, wm)
    return _march_tets(pos, tet_fx4, sdf)


# revision 8
# speedup vs baseline: 1.0460x; 1.0049x over previous
"""DMTetGeometry kernel for 8 Trainium2 NeuronCores.

Split of work:
  - device (8 cores, data-parallel over vertices): the 5-layer SDF MLP
    (27 -> 256 -> 256 -> 256 -> 256 -> 1), which is all of the FLOPs.
    Activations are kept feature-major ([features, rows]) so every layer
    is a plain K-on-partitions matmul with zero transposes.
  - host: positional encoding (sin/cos must match the fp32 reference
    closely; the ACT engine's table-based Sin with fp32 range reduction
    is not accurate enough for the sign-critical sdf), and the marching
    tetrahedra stage (data-dependent shapes: unique/cumsum/masking).

The output's discrete structure depends on sign(sdf), so the MLP is run
in full fp32 on the PE array.
"""

import sys

for _p in ("/root/.axon_site/_ro/trn_rl_repo", "/opt/trn_rl_repo"):
    if _p not in sys.path:
        sys.path.append(_p)

import numpy as np

import concourse.bacc as bacc
import concourse.mybir as mybir
import concourse.tile as tile
from concourse.bass_utils import run_bass_kernel_spmd

N_CORES = 8
N_VERTS = 200000
PER_CORE = 25000
BLK = 512
NBLK = (PER_CORE + BLK - 1) // BLK  # 49
PER_CORE_PAD = NBLK * BLK  # 25088
D_IN = 27
HID = 256
FREQ_NUM = 4

_TRI_TABLE = np.array([
    [-1,-1,-1,-1,-1,-1],[1,0,2,-1,-1,-1],[4,0,3,-1,-1,-1],[1,4,2,1,3,4],
    [3,1,5,-1,-1,-1],[2,3,0,2,5,3],[1,4,0,1,5,4],[4,2,5,-1,-1,-1],
    [4,5,2,-1,-1,-1],[4,1,0,4,5,1],[3,2,0,3,5,2],[1,3,5,-1,-1,-1],
    [4,1,2,4,3,1],[3,0,4,-1,-1,-1],[2,0,1,-1,-1,-1],[-1,-1,-1,-1,-1,-1]], dtype=np.int32)
_NUM_TRI = np.array([0,1,1,2,1,2,2,1,1,2,2,1,2,1,1,0], dtype=np.int32)
_BASE_EDGES = np.array([0,1,0,2,0,3,1,2,1,3,2,3], dtype=np.int32)

F32 = mybir.dt.float32
F32R = mybir.dt.float32r
Relu = mybir.ActivationFunctionType.Relu
Identity = mybir.ActivationFunctionType.Identity
USE_F32R = False


def _mm(ap):
    return ap.bitcast(F32R) if USE_F32R else ap


def _build_nc(nblk=NBLK):
    n_cols = nblk * BLK
    nc = bacc.Bacc("TRN2", target_bir_lowering=False, debug=False,
                   enable_asserts=False)
    x = nc.dram_tensor("x", [D_IN, n_cols], F32, kind="ExternalInput")
    w0 = nc.dram_tensor("w0", [D_IN, HID], F32, kind="ExternalInput")
    # hidden weights pre-chunked on host: [k_chunk, 128, 256]
    wh = [nc.dram_tensor(f"w{l}c", [2, 128, HID], F32, kind="ExternalInput")
          for l in (1, 2, 3)]
    # biases pre-chunked on host: [128, 2]
    bs = [nc.dram_tensor(f"b{l}c", [128, 2], F32, kind="ExternalInput")
          for l in (0, 1, 2, 3)]
    wf = nc.dram_tensor("wfc", [128, 2], F32, kind="ExternalInput")
    bf = nc.dram_tensor("bfc", [1, 1], F32, kind="ExternalInput")
    sdf = nc.dram_tensor("sdf", [1, n_cols], F32, kind="ExternalOutput")

    with tile.TileContext(nc) as tc:
        with (
            tc.tile_pool(name="consts", bufs=1) as cpool,
            tc.tile_pool(name="xin", bufs=4) as xpool,
            tc.tile_pool(name="acts", bufs=10) as hpool,
            tc.tile_pool(name="souts", bufs=4) as spool,
            tc.tile_pool(name="ps", bufs=6, space="PSUM") as ppool,
            tc.tile_pool(name="psf", bufs=2, space="PSUM") as pfpool,
        ):
            w0_sb = cpool.tile([D_IN, HID], F32, tag="w0")
            nc.sync.dma_start(out=w0_sb[:], in_=w0[:, :])
            wh_sb = []
            for li, w in enumerate(wh):
                pair = []
                for k in range(2):
                    t = cpool.tile([128, HID], F32, tag=f"w{li}{k}")
                    nc.sync.dma_start(out=t[:], in_=w[k])
                    pair.append(t)
                wh_sb.append(pair)
            bs_sb = []
            for li, b in enumerate(bs):
                t = cpool.tile([128, 2], F32, tag=f"b{li}")
                nc.sync.dma_start(out=t[:], in_=b[:, :])
                bs_sb.append(t)
            wf_sb = cpool.tile([128, 2], F32, tag="wf")
            nc.sync.dma_start(out=wf_sb[:], in_=wf[:, :])
            bf_sb = cpool.tile([1, 1], F32, tag="bf")
            nc.sync.dma_start(out=bf_sb[:], in_=bf[:, :])

            # HAM warm-up: keep the PE busy during the weight-DMA
            # preamble so the clock gate is at 2.4 GHz when real work
            # arrives (~3.4 us of sustained activity flips it).
            warm = cpool.tile([128, 128], F32, tag="warm")
            nc.gpsimd.memset(warm[:], 0.0)
            wps = pfpool.tile([128, 64], F32, tag="psf")
            for _ in range(9):
                nc.tensor.matmul(wps[:], lhsT=_mm(warm[:]),
                                 rhs=_mm(warm[:, :64]), start=True, stop=True)

            def bias_relu(h, ps, b_ap, use_act):
                if use_act:
                    nc.scalar.activation(h[:], ps[:], Relu, bias=b_ap)
                else:
                    nc.vector.tensor_scalar(
                        h[:], ps[:], b_ap, 0.0,
                        mybir.AluOpType.add, mybir.AluOpType.max)

            # process blocks in pairs so consecutive matmuls share the
            # same stationary (lhsT) operand
            for i0 in range(0, nblk, 2):
                blks = [i0] if i0 + 1 >= nblk else [i0, i0 + 1]
                nb = len(blks)
                cols = [slice(i * BLK, (i + 1) * BLK) for i in blks]
                xts = []
                for b in range(nb):
                    xt = xpool.tile([D_IN, BLK], F32, tag="xt")
                    nc.sync.dma_start(out=xt[:], in_=x[:, cols[b]])
                    xts.append(xt)

                # layer 0: 27 -> 256
                pss = {}
                for m in range(2):
                    for b in range(nb):
                        ps = ppool.tile([128, BLK], F32, tag="ps")
                        nc.tensor.matmul(
                            ps[:], lhsT=_mm(w0_sb[:, m * 128:(m + 1) * 128]),
                            rhs=_mm(xts[b][:]), start=True, stop=True)
                        pss[(b, m)] = ps
                hcur = [[None, None] for _ in range(nb)]
                for m in range(2):
                    for b in range(nb):
                        h = hpool.tile([128, BLK], F32, tag="h")
                        bias_relu(h, pss[(b, m)], bs_sb[0][:, m:m + 1],
                                  use_act=(m == 0))
                        hcur[b][m] = h

                # layers 1..3: 256 -> 256
                for li in range(3):
                    pss = {}
                    for m in range(2):
                        for k in range(2):
                            for b in range(nb):
                                if k == 0:
                                    ps = ppool.tile([128, BLK], F32, tag="ps")
                                    pss[(b, m)] = ps
                                nc.tensor.matmul(
                                    pss[(b, m)][:],
                                    lhsT=_mm(wh_sb[li][k][:, m * 128:(m + 1) * 128]),
                                    rhs=_mm(hcur[b][k][:]),
                                    start=(k == 0), stop=(k == 1))
                    hnew = [[None, None] for _ in range(nb)]
                    for m in range(2):
                        for b in range(nb):
                            h = hpool.tile([128, BLK], F32, tag="h")
                            bias_relu(h, pss[(b, m)], bs_sb[li + 1][:, m:m + 1],
                                      use_act=(m == 0))
                            hnew[b][m] = h
                    hcur = hnew

                # final: 256 -> 1
                psfs = []
                for k in range(2):
                    for b in range(nb):
                        if k == 0:
                            psfs.append(pfpool.tile([1, BLK], F32, tag="psf",
                                                    name="psf"))
                        nc.tensor.matmul(psfs[b][:], lhsT=_mm(wf_sb[:, k:k + 1]),
                                         rhs=_mm(hcur[b][k][:]),
                                         start=(k == 0), stop=(k == 1))
                for b in range(nb):
                    so = spool.tile([1, BLK], F32, tag="so")
                    nc.scalar.activation(so[:], psfs[b][:], Identity,
                                         bias=bf_sb[0:1, 0:1])
                    nc.sync.dma_start(out=sdf[:, cols[b]], in_=so[:])

    nc.compile()
    return nc


_NC_CACHE = {}


def _get_nc():
    if "nc" not in _NC_CACHE:
        _NC_CACHE["nc"] = _build_nc()
    return _NC_CACHE["nc"]


def _posenc_t(pos):
    """Feature-major positional encoding [27, N], fp32, matching the
    reference's fp32 elementwise ops."""
    n = pos.shape[0]
    posT = np.ascontiguousarray(pos.T.astype(np.float32, copy=False))  # [3, N]
    x = np.empty((D_IN, n), np.float32)
    x[0:3] = posT
    for i in range(FREQ_NUM):
        f = np.float32(float(2 ** i) * np.pi)
        fx = f * posT
        x[3 + 6 * i: 6 + 6 * i] = np.sin(fx)
        x[6 + 6 * i: 9 + 6 * i] = np.cos(fx)
    return x


def _chunk_inputs(w0, b0, w1, b1, w2, b2, w3, b3, wf, bf):
    f32 = lambda a: np.ascontiguousarray(np.asarray(a), dtype=np.float32)
    out = {"w0": f32(w0)}
    for name, w in (("w1c", w1), ("w2c", w2), ("w3c", w3)):
        w = f32(w)
        out[name] = np.ascontiguousarray(np.stack([w[:128], w[128:]]))
    for name, b in (("b0c", b0), ("b1c", b1), ("b2c", b2), ("b3c", b3)):
        b = f32(b)
        out[name] = np.ascontiguousarray(b.reshape(2, 128).T)
    wf = f32(wf).reshape(-1)
    out["wfc"] = np.ascontiguousarray(wf.reshape(2, 128).T)
    out["bfc"] = f32(bf).reshape(1, 1)
    return out


def _sdf_device(x_t, weight_maps):
    """x_t: [27, N_VERTS] fp32 -> sdf [N_VERTS] fp32 via 8-core SPMD."""
    nc = _get_nc()
    in_maps = []
    for c in range(N_CORES):
        xc = np.zeros((D_IN, PER_CORE_PAD), np.float32)
        xc[:, :PER_CORE] = x_t[:, c * PER_CORE:(c + 1) * PER_CORE]
        m = {"x": xc}
        m.update(weight_maps)
        in_maps.append(m)
    res = run_bass_kernel_spmd(nc, in_maps, list(range(N_CORES)))
    sdf = np.empty(N_VERTS, np.float32)
    for c in range(N_CORES):
        sdf[c * PER_CORE:(c + 1) * PER_CORE] = \
            res.results[c]["sdf"][0, :PER_CORE]
    return sdf


def _march_tets(pos, tet_fx4, sdf):
    """Marching tetrahedra, numpy mirror of the jnp reference."""
    occ = sdf > 0
    occ_fx4 = occ[tet_fx4]                       # [F,4]
    occ_sum = occ_fx4.sum(-1)
    valid = (occ_sum > 0) & (occ_sum < 4)
    vt = tet_fx4[valid]                          # [V,4]
    occ_v = occ_fx4[valid]

    e = vt[:, _BASE_EDGES].reshape(-1, 2)        # [6V,2]
    lo = np.minimum(e[:, 0], e[:, 1]).astype(np.int64)
    hi = np.maximum(e[:, 0], e[:, 1]).astype(np.int64)
    key = lo * N_VERTS + hi
    uniq_key, inv = np.unique(key, return_inverse=True)
    u_lo = (uniq_key // N_VERTS).astype(np.int64)
    u_hi = (uniq_key % N_VERTS).astype(np.int64)

    cross = (occ[u_lo].astype(np.int32) + occ[u_hi].astype(np.int32)) == 1
    mapping = np.where(cross, np.cumsum(cross, dtype=np.int64) - 1, -1)
    idx_map = mapping[inv].reshape(-1, 6).astype(np.int32)

    ilo = u_lo[cross]
    ihi = u_hi[cross]
    p0 = pos[ilo]
    p1 = pos[ihi]
    s0 = sdf[ilo]
    s1 = sdf[ihi]
    denom = s0 - s1
    verts = (p0 * (-s1)[:, None] + p1 * s0[:, None]) / denom[:, None]

    tetindex = (occ_v.astype(np.int32) *
                np.array([1, 2, 4, 8], np.int32)).sum(-1)
    ntri = _NUM_TRI[tetindex]
    tri = _TRI_TABLE[tetindex]
    m1 = ntri == 1
    m2 = ntri == 2
    f1 = np.take_along_axis(idx_map[m1], tri[m1][:, :3], axis=1).reshape(-1, 3)
    f2 = np.take_along_axis(idx_map[m2], tri[m2][:, :6], axis=1).reshape(-1, 3)
    faces = np.concatenate([f1, f2], axis=0).astype(np.int32)
    return verts.astype(np.float32), faces


def kernel(pos, tet_fx4, w0, b0, w1, b1, w2, b2, w3, b3, wf, bf):
    pos = np.ascontiguousarray(np.asarray(pos), dtype=np.float32)
    tet_fx4 = np.ascontiguousarray(np.asarray(tet_fx4), dtype=np.int32)
    x_t = _posenc_t(pos)
    wm = _chunk_inputs(w0, b0, w1, b1, w2, b2, w3, b3, wf, bf)
    sdf = _sdf_device(x_t, wm)
    return _march_tets(pos, tet_fx4, sdf)


# revision 11
# speedup vs baseline: 1.0556x; 1.0092x over previous
"""DMTetGeometry kernel for 8 Trainium2 NeuronCores.

Split of work:
  - device (8 cores, data-parallel over vertices): the 5-layer SDF MLP
    (27 -> 256 -> 256 -> 256 -> 256 -> 1), which is all of the FLOPs.
    Activations are kept feature-major ([features, rows]) so every layer
    is a plain K-on-partitions matmul with zero transposes.
  - host: positional encoding (sin/cos must match the fp32 reference
    closely; the ACT engine's table-based Sin with fp32 range reduction
    is not accurate enough for the sign-critical sdf), and the marching
    tetrahedra stage (data-dependent shapes: unique/cumsum/masking).

The output's discrete structure depends on sign(sdf), so the MLP is run
in full fp32 on the PE array.
"""

import sys

for _p in ("/root/.axon_site/_ro/trn_rl_repo", "/opt/trn_rl_repo"):
    if _p not in sys.path:
        sys.path.append(_p)

import numpy as np

import concourse.bacc as bacc
import concourse.mybir as mybir
import concourse.tile as tile
from concourse.bass_utils import run_bass_kernel_spmd

N_CORES = 8
N_VERTS = 200000
PER_CORE = 25000
BLK = 512
NBLK = (PER_CORE + BLK - 1) // BLK  # 49
PER_CORE_PAD = NBLK * BLK  # 25088
D_IN = 27
D_PAD = 32  # layer-0 K padded with zero rows (bitwise no-op on the sums)
HID = 256
FREQ_NUM = 4

_TRI_TABLE = np.array([
    [-1,-1,-1,-1,-1,-1],[1,0,2,-1,-1,-1],[4,0,3,-1,-1,-1],[1,4,2,1,3,4],
    [3,1,5,-1,-1,-1],[2,3,0,2,5,3],[1,4,0,1,5,4],[4,2,5,-1,-1,-1],
    [4,5,2,-1,-1,-1],[4,1,0,4,5,1],[3,2,0,3,5,2],[1,3,5,-1,-1,-1],
    [4,1,2,4,3,1],[3,0,4,-1,-1,-1],[2,0,1,-1,-1,-1],[-1,-1,-1,-1,-1,-1]], dtype=np.int32)
_NUM_TRI = np.array([0,1,1,2,1,2,2,1,1,2,2,1,2,1,1,0], dtype=np.int32)
_BASE_EDGES = np.array([0,1,0,2,0,3,1,2,1,3,2,3], dtype=np.int32)

F32 = mybir.dt.float32
F32R = mybir.dt.float32r
Relu = mybir.ActivationFunctionType.Relu
Identity = mybir.ActivationFunctionType.Identity
USE_F32R = False


def _mm(ap):
    return ap.bitcast(F32R) if USE_F32R else ap


def _build_nc(nblk=NBLK):
    n_cols = nblk * BLK
    nc = bacc.Bacc("TRN2", target_bir_lowering=False, debug=False,
                   enable_asserts=False)
    x = nc.dram_tensor("x", [D_PAD, n_cols], F32, kind="ExternalInput")
    w0 = nc.dram_tensor("w0", [D_PAD, HID], F32, kind="ExternalInput")
    # hidden weights pre-chunked on host: [k_chunk, 128, 256]
    wh = [nc.dram_tensor(f"w{l}c", [2, 128, HID], F32, kind="ExternalInput")
          for l in (1, 2, 3)]
    # biases pre-chunked on host: [128, 2]
    bs = [nc.dram_tensor(f"b{l}c", [128, 2], F32, kind="ExternalInput")
          for l in (0, 1, 2, 3)]
    wf = nc.dram_tensor("wfc", [128, 2], F32, kind="ExternalInput")
    bf = nc.dram_tensor("bfc", [1, 1], F32, kind="ExternalInput")
    sdf = nc.dram_tensor("sdf", [1, n_cols], F32, kind="ExternalOutput")

    with tile.TileContext(nc) as tc:
        with (
            tc.tile_pool(name="consts", bufs=1) as cpool,
            tc.tile_pool(name="xin", bufs=4) as xpool,
            tc.tile_pool(name="acts", bufs=10) as hpool,
            tc.tile_pool(name="souts", bufs=4) as spool,
            tc.tile_pool(name="ps", bufs=6, space="PSUM") as ppool,
            tc.tile_pool(name="psf", bufs=2, space="PSUM") as pfpool,
        ):
            # HAM warm-up + first-pair input + layer-0 consts go FIRST so
            # the PE has real work within ~2 us; remaining weights stream in
            # behind them.
            warm = cpool.tile([128, 128], F32, tag="warm")
            nc.gpsimd.memset(warm[:], 0.0)
            wps = pfpool.tile([128, 64], F32, tag="psf", name="wps")
            for _ in range(10):
                nc.tensor.matmul(wps[:], lhsT=_mm(warm[:]),
                                 rhs=_mm(warm[:, :64]), start=True, stop=True)

            xt0 = xpool.tile([D_PAD, 2 * BLK], F32, tag="xt")
            nc.sync.dma_start(out=xt0[:], in_=x[:, 0:2 * BLK])
            w0_sb = cpool.tile([D_PAD, HID], F32, tag="w0")
            nc.sync.dma_start(out=w0_sb[:], in_=w0[:, :])
            wh_sb = []
            for li, w in enumerate(wh):
                pair = []
                for k in range(2):
                    t = cpool.tile([128, HID], F32, tag=f"w{li}{k}")
                    nc.sync.dma_start(out=t[:], in_=w[k])
                    pair.append(t)
                wh_sb.append(pair)
            bs_sb = []
            for li, b in enumerate(bs):
                t = cpool.tile([128, 2], F32, tag=f"b{li}")
                nc.sync.dma_start(out=t[:], in_=b[:, :])
                bs_sb.append(t)
            wf_sb = cpool.tile([128, 2], F32, tag="wf")
            nc.sync.dma_start(out=wf_sb[:], in_=wf[:, :])
            bf_sb = cpool.tile([1, 1], F32, tag="bf")
            nc.sync.dma_start(out=bf_sb[:], in_=bf[:, :])

            def bias_relu(h, ps, b_ap, use_act):
                if use_act:
                    nc.scalar.activation(h[:], ps[:], Relu, bias=b_ap)
                else:
                    nc.vector.tensor_scalar(
                        h[:], ps[:], b_ap, 0.0,
                        mybir.AluOpType.add, mybir.AluOpType.max)

            # process blocks in pairs so consecutive matmuls share the
            # same stationary (lhsT) operand
            for i0 in range(0, nblk, 2):
                blks = [i0] if i0 + 1 >= nblk else [i0, i0 + 1]
                nb = len(blks)
                cols = [slice(i * BLK, (i + 1) * BLK) for i in blks]
                if i0 == 0:
                    xt = xt0
                else:
                    xt = xpool.tile([D_PAD, nb * BLK], F32, tag="xt")
                    nc.sync.dma_start(
                        out=xt[:], in_=x[:, i0 * BLK:(i0 + nb) * BLK])
                xts = [xt[:, b * BLK:(b + 1) * BLK] for b in range(nb)]

                # layer 0: 27 -> 256
                pss = {}
                for m in range(2):
                    for b in range(nb):
                        ps = ppool.tile([128, BLK], F32, tag="ps")
                        nc.tensor.matmul(
                            ps[:], lhsT=_mm(w0_sb[:, m * 128:(m + 1) * 128]),
                            rhs=_mm(xts[b]), start=True, stop=True)
                        pss[(b, m)] = ps
                hcur = [[None, None] for _ in range(nb)]
                for m in range(2):
                    for b in range(nb):
                        h = hpool.tile([128, BLK], F32, tag="h")
                        bias_relu(h, pss[(b, m)], bs_sb[0][:, m:m + 1],
                                  use_act=(m == 0))
                        hcur[b][m] = h

                # layers 1..3: 256 -> 256
                for li in range(3):
                    pss = {}
                    for m in range(2):
                        for k in range(2):
                            for b in range(nb):
                                if k == 0:
                                    ps = ppool.tile([128, BLK], F32, tag="ps")
                                    pss[(b, m)] = ps
                                nc.tensor.matmul(
                                    pss[(b, m)][:],
                                    lhsT=_mm(wh_sb[li][k][:, m * 128:(m + 1) * 128]),
                                    rhs=_mm(hcur[b][k][:]),
                                    start=(k == 0), stop=(k == 1))
                    hnew = [[None, None] for _ in range(nb)]
                    for m in range(2):
                        for b in range(nb):
                            h = hpool.tile([128, BLK], F32, tag="h")
                            bias_relu(h, pss[(b, m)], bs_sb[li + 1][:, m:m + 1],
                                      use_act=(m == 0))
                            hnew[b][m] = h
                    hcur = hnew

                # final: 256 -> 1
                psfs = []
                for k in range(2):
                    for b in range(nb):
                        if k == 0:
                            psfs.append(pfpool.tile([1, BLK], F32, tag="psf",
                                                    name="psf"))
                        nc.tensor.matmul(psfs[b][:], lhsT=_mm(wf_sb[:, k:k + 1]),
                                         rhs=_mm(hcur[b][k][:]),
                                         start=(k == 0), stop=(k == 1))
                for b in range(nb):
                    so = spool.tile([1, BLK], F32, tag="so")
                    nc.scalar.activation(so[:], psfs[b][:], Identity,
                                         bias=bf_sb[0:1, 0:1])
                    nc.sync.dma_start(out=sdf[:, cols[b]], in_=so[:])

    nc.compile()
    return nc


_NC_CACHE = {}


def _get_nc():
    if "nc" not in _NC_CACHE:
        _NC_CACHE["nc"] = _build_nc()
    return _NC_CACHE["nc"]


def _posenc_t(pos):
    """Feature-major positional encoding [27, N], fp32, matching the
    reference's fp32 elementwise ops."""
    n = pos.shape[0]
    posT = np.ascontiguousarray(pos.T.astype(np.float32, copy=False))  # [3, N]
    x = np.zeros((D_PAD, n), np.float32)
    x[0:3] = posT
    for i in range(FREQ_NUM):
        f = np.float32(float(2 ** i) * np.pi)
        fx = f * posT
        x[3 + 6 * i: 6 + 6 * i] = np.sin(fx)
        x[6 + 6 * i: 9 + 6 * i] = np.cos(fx)
    return x


def _chunk_inputs(w0, b0, w1, b1, w2, b2, w3, b3, wf, bf):
    f32 = lambda a: np.ascontiguousarray(np.asarray(a), dtype=np.float32)
    w0p = np.zeros((D_PAD, HID), np.float32)
    w0p[:D_IN] = f32(w0)
    out = {"w0": w0p}
    for name, w in (("w1c", w1), ("w2c", w2), ("w3c", w3)):
        w = f32(w)
        out[name] = np.ascontiguousarray(np.stack([w[:128], w[128:]]))
    for name, b in (("b0c", b0), ("b1c", b1), ("b2c", b2), ("b3c", b3)):
        b = f32(b)
        out[name] = np.ascontiguousarray(b.reshape(2, 128).T)
    wf = f32(wf).reshape(-1)
    out["wfc"] = np.ascontiguousarray(wf.reshape(2, 128).T)
    out["bfc"] = f32(bf).reshape(1, 1)
    return out


def _sdf_device(x_t, weight_maps):
    """x_t: [27, N_VERTS] fp32 -> sdf [N_VERTS] fp32 via 8-core SPMD."""
    nc = _get_nc()
    in_maps = []
    for c in range(N_CORES):
        xc = np.zeros((D_PAD, PER_CORE_PAD), np.float32)
        xc[:, :PER_CORE] = x_t[:, c * PER_CORE:(c + 1) * PER_CORE]
        m = {"x": xc}
        m.update(weight_maps)
        in_maps.append(m)
    res = run_bass_kernel_spmd(nc, in_maps, list(range(N_CORES)))
    sdf = np.empty(N_VERTS, np.float32)
    for c in range(N_CORES):
        sdf[c * PER_CORE:(c + 1) * PER_CORE] = \
            res.results[c]["sdf"][0, :PER_CORE]
    return sdf


def _march_tets(pos, tet_fx4, sdf):
    """Marching tetrahedra, numpy mirror of the jnp reference."""
    occ = sdf > 0
    occ_fx4 = occ[tet_fx4]                       # [F,4]
    occ_sum = occ_fx4.sum(-1)
    valid = (occ_sum > 0) & (occ_sum < 4)
    vt = tet_fx4[valid]                          # [V,4]
    occ_v = occ_fx4[valid]

    e = vt[:, _BASE_EDGES].reshape(-1, 2)        # [6V,2]
    lo = np.minimum(e[:, 0], e[:, 1]).astype(np.int64)
    hi = np.maximum(e[:, 0], e[:, 1]).astype(np.int64)
    key = lo * N_VERTS + hi
    uniq_key, inv = np.unique(key, return_inverse=True)
    u_lo = (uniq_key // N_VERTS).astype(np.int64)
    u_hi = (uniq_key % N_VERTS).astype(np.int64)

    cross = (occ[u_lo].astype(np.int32) + occ[u_hi].astype(np.int32)) == 1
    mapping = np.where(cross, np.cumsum(cross, dtype=np.int64) - 1, -1)
    idx_map = mapping[inv].reshape(-1, 6).astype(np.int32)

    ilo = u_lo[cross]
    ihi = u_hi[cross]
    p0 = pos[ilo]
    p1 = pos[ihi]
    s0 = sdf[ilo]
    s1 = sdf[ihi]
    denom = s0 - s1
    verts = (p0 * (-s1)[:, None] + p1 * s0[:, None]) / denom[:, None]

    tetindex = (occ_v.astype(np.int32) *
                np.array([1, 2, 4, 8], np.int32)).sum(-1)
    ntri = _NUM_TRI[tetindex]
    tri = _TRI_TABLE[tetindex]
    m1 = ntri == 1
    m2 = ntri == 2
    f1 = np.take_along_axis(idx_map[m1], tri[m1][:, :3], axis=1).reshape(-1, 3)
    f2 = np.take_along_axis(idx_map[m2], tri[m2][:, :6], axis=1).reshape(-1, 3)
    faces = np.concatenate([f1, f2], axis=0).astype(np.int32)
    return verts.astype(np.float32), faces


def kernel(pos, tet_fx4, w0, b0, w1, b1, w2, b2, w3, b3, wf, bf):
    pos = np.ascontiguousarray(np.asarray(pos), dtype=np.float32)
    tet_fx4 = np.ascontiguousarray(np.asarray(tet_fx4), dtype=np.int32)
    x_t = _posenc_t(pos)
    wm = _chunk_inputs(w0, b0, w1, b1, w2, b2, w3, b3, wf, bf)
    sdf = _sdf_device(x_t, wm)
    return _march_tets(pos, tet_fx4, sdf)


# revision 12
# speedup vs baseline: 1.1224x; 1.0633x over previous
"""DMTetGeometry kernel for 8 Trainium2 NeuronCores.

Split of work:
  - device (8 cores, data-parallel over vertices): the 5-layer SDF MLP
    (27 -> 256 -> 256 -> 256 -> 256 -> 1), which is all of the FLOPs.
    Activations are kept feature-major ([features, rows]) so every layer
    is a plain K-on-partitions matmul with zero transposes.
  - host: positional encoding (sin/cos must match the fp32 reference
    closely; the ACT engine's table-based Sin with fp32 range reduction
    is not accurate enough for the sign-critical sdf), and the marching
    tetrahedra stage (data-dependent shapes: unique/cumsum/masking).

The output's discrete structure depends on sign(sdf), so the MLP is run
in full fp32 on the PE array.
"""

import sys

for _p in ("/root/.axon_site/_ro/trn_rl_repo", "/opt/trn_rl_repo"):
    if _p not in sys.path:
        sys.path.append(_p)

import numpy as np

import concourse.bacc as bacc
import concourse.mybir as mybir
import concourse.tile as tile
from concourse.bass_utils import run_bass_kernel_spmd

N_CORES = 8
N_VERTS = 200000
PER_CORE = 25000
BLK = 512
NBLK = (PER_CORE + BLK - 1) // BLK  # 49
PER_CORE_PAD = NBLK * BLK  # 25088
D_IN = 27
D_PAD = 32  # layer-0 K padded with zero rows (bitwise no-op on the sums)
HID = 256
FREQ_NUM = 4

_TRI_TABLE = np.array([
    [-1,-1,-1,-1,-1,-1],[1,0,2,-1,-1,-1],[4,0,3,-1,-1,-1],[1,4,2,1,3,4],
    [3,1,5,-1,-1,-1],[2,3,0,2,5,3],[1,4,0,1,5,4],[4,2,5,-1,-1,-1],
    [4,5,2,-1,-1,-1],[4,1,0,4,5,1],[3,2,0,3,5,2],[1,3,5,-1,-1,-1],
    [4,1,2,4,3,1],[3,0,4,-1,-1,-1],[2,0,1,-1,-1,-1],[-1,-1,-1,-1,-1,-1]], dtype=np.int32)
_NUM_TRI = np.array([0,1,1,2,1,2,2,1,1,2,2,1,2,1,1,0], dtype=np.int32)
_BASE_EDGES = np.array([0,1,0,2,0,3,1,2,1,3,2,3], dtype=np.int32)

F32 = mybir.dt.float32
F32R = mybir.dt.float32r
Relu = mybir.ActivationFunctionType.Relu
Identity = mybir.ActivationFunctionType.Identity
USE_F32R = False


def _mm(ap):
    return ap.bitcast(F32R) if USE_F32R else ap


def _build_nc(nblk=NBLK):
    n_cols = nblk * BLK
    nc = bacc.Bacc("TRN2", target_bir_lowering=False, debug=False,
                   enable_asserts=False)
    x = nc.dram_tensor("x", [D_PAD, n_cols], F32, kind="ExternalInput")
    w0 = nc.dram_tensor("w0", [D_PAD, HID], F32, kind="ExternalInput")
    # hidden weights pre-chunked on host: [k_chunk, 128, 256]
    wh = [nc.dram_tensor(f"w{l}c", [2, 128, HID], F32, kind="ExternalInput")
          for l in (1, 2, 3)]
    # biases pre-chunked on host: [128, 2]
    bs = [nc.dram_tensor(f"b{l}c", [128, 2], F32, kind="ExternalInput")
          for l in (0, 1, 2, 3)]
    wf = nc.dram_tensor("wfc", [128, 2], F32, kind="ExternalInput")
    bf = nc.dram_tensor("bfc", [1, 1], F32, kind="ExternalInput")
    sdf = nc.dram_tensor("sdf", [1, n_cols], F32, kind="ExternalOutput")

    with tile.TileContext(nc) as tc:
        with (
            tc.tile_pool(name="consts", bufs=1) as cpool,
            tc.tile_pool(name="xin", bufs=4) as xpool,
            tc.tile_pool(name="acts", bufs=10) as hpool,
            tc.tile_pool(name="souts", bufs=4) as spool,
            tc.tile_pool(name="ps", bufs=6, space="PSUM") as ppool,
            tc.tile_pool(name="psf", bufs=2, space="PSUM") as pfpool,
        ):
            # HAM warm-up + first-pair input + layer-0 consts go FIRST so
            # the PE has real work within ~2 us; remaining weights stream in
            # behind them.
            warm = cpool.tile([128, 128], F32, tag="warm")
            nc.gpsimd.memset(warm[:], 0.0)
            wps = pfpool.tile([128, 64], F32, tag="psf", name="wps")
            for _ in range(10):
                nc.tensor.matmul(wps[:], lhsT=_mm(warm[:]),
                                 rhs=_mm(warm[:, :64]), start=True, stop=True)

            xt0 = xpool.tile([D_PAD, 2 * BLK], F32, tag="xt")
            nc.sync.dma_start(out=xt0[:], in_=x[:, 0:2 * BLK])
            w0_sb = cpool.tile([D_PAD, HID], F32, tag="w0")
            nc.sync.dma_start(out=w0_sb[:], in_=w0[:, :])
            wh_sb = []
            for li, w in enumerate(wh):
                pair = []
                for k in range(2):
                    t = cpool.tile([128, HID], F32, tag=f"w{li}{k}")
                    nc.sync.dma_start(out=t[:], in_=w[k])
                    pair.append(t)
                wh_sb.append(pair)
            bs_sb = []
            for li, b in enumerate(bs):
                t = cpool.tile([128, 2], F32, tag=f"b{li}")
                nc.sync.dma_start(out=t[:], in_=b[:, :])
                bs_sb.append(t)
            wf_sb = cpool.tile([128, 2], F32, tag="wf")
            nc.sync.dma_start(out=wf_sb[:], in_=wf[:, :])
            bf_sb = cpool.tile([1, 1], F32, tag="bf")
            nc.sync.dma_start(out=bf_sb[:], in_=bf[:, :])
            ones_sb = cpool.tile([128, 1], F32, tag="ones")
            nc.gpsimd.memset(ones_sb[:], 1.0)

            def bias_relu(h, ps, b_ap, use_act):
                if use_act:
                    nc.scalar.activation(h[:], ps[:], Relu, bias=b_ap)
                else:
                    nc.vector.tensor_scalar(
                        h[:], ps[:], b_ap, 0.0,
                        mybir.AluOpType.add, mybir.AluOpType.max)

            # process blocks in pairs so consecutive matmuls share the
            # same stationary (lhsT) operand
            for i0 in range(0, nblk, 2):
                blks = [i0] if i0 + 1 >= nblk else [i0, i0 + 1]
                nb = len(blks)
                cols = [slice(i * BLK, (i + 1) * BLK) for i in blks]
                if i0 == 0:
                    xt = xt0
                else:
                    xt = xpool.tile([D_PAD, nb * BLK], F32, tag="xt")
                    nc.sync.dma_start(
                        out=xt[:], in_=x[:, i0 * BLK:(i0 + nb) * BLK])
                xts = [xt[:, b * BLK:(b + 1) * BLK] for b in range(nb)]

                # layer 0: 27 -> 256
                pss = {}
                for m in range(2):
                    for b in range(nb):
                        ps = ppool.tile([128, BLK], F32, tag="ps")
                        nc.tensor.matmul(
                            ps[:], lhsT=_mm(w0_sb[:, m * 128:(m + 1) * 128]),
                            rhs=_mm(xts[b]), start=True, stop=True)
                        pss[(b, m)] = ps
                hcur = [[None, None] for _ in range(nb)]
                for m in range(2):
                    for b in range(nb):
                        h = hpool.tile([128, BLK], F32, tag="h")
                        bias_relu(h, pss[(b, m)], bs_sb[0][:, m:m + 1],
                                  use_act=(m == 0))
                        hcur[b][m] = h

                # layers 1..3: 256 -> 256
                for li in range(3):
                    pss = {}
                    for m in range(2):
                        for k in range(2):
                            for b in range(nb):
                                if k == 0:
                                    ps = ppool.tile([128, BLK], F32, tag="ps")
                                    pss[(b, m)] = ps
                                nc.tensor.matmul(
                                    pss[(b, m)][:],
                                    lhsT=_mm(wh_sb[li][k][:, m * 128:(m + 1) * 128]),
                                    rhs=_mm(hcur[b][k][:]),
                                    start=(k == 0), stop=(k == 1))
                    hnew = [[None, None] for _ in range(nb)]
                    for m in range(2):
                        for b in range(nb):
                            h = hpool.tile([128, BLK], F32, tag="h")
                            bias_relu(h, pss[(b, m)], bs_sb[li + 1][:, m:m + 1],
                                      use_act=(m == 0))
                            hnew[b][m] = h
                    hcur = hnew

                # final: 256 -> 1.  Fold wf into the activations
                # elementwise (per-partition scale) and reduce the 128
                # partitions with a constant ones matmul: one PE pass per
                # block instead of two.
                psfs = []
                for b in range(nb):
                    t1 = hpool.tile([128, BLK], F32, tag="zf", name="t1")
                    nc.scalar.activation(t1[:], hcur[b][0][:],
                                         mybir.ActivationFunctionType.Copy,
                                         scale=wf_sb[:, 0:1])
                    t2 = hpool.tile([128, BLK], F32, tag="zf", name="t2")
                    nc.vector.tensor_scalar(t2[:], hcur[b][1][:],
                                            wf_sb[:, 1:2], None,
                                            mybir.AluOpType.mult)
                    z = hpool.tile([128, BLK], F32, tag="zf", name="z")
                    nc.vector.tensor_tensor(z[:], t1[:], t2[:],
                                            mybir.AluOpType.add)
                    psf = pfpool.tile([1, BLK], F32, tag="psf", name="psf")
                    nc.tensor.matmul(psf[:], lhsT=_mm(ones_sb[:]),
                                     rhs=_mm(z[:]), start=True, stop=True)
                    psfs.append(psf)
                for b in range(nb):
                    so = spool.tile([1, BLK], F32, tag="so")
                    nc.scalar.activation(so[:], psfs[b][:], Identity,
                                         bias=bf_sb[0:1, 0:1])
                    nc.sync.dma_start(out=sdf[:, cols[b]], in_=so[:])

    nc.compile()
    return nc


_NC_CACHE = {}


def _get_nc():
    if "nc" not in _NC_CACHE:
        _NC_CACHE["nc"] = _build_nc()
    return _NC_CACHE["nc"]


def _posenc_t(pos):
    """Feature-major positional encoding [27, N], fp32, matching the
    reference's fp32 elementwise ops."""
    n = pos.shape[0]
    posT = np.ascontiguousarray(pos.T.astype(np.float32, copy=False))  # [3, N]
    x = np.zeros((D_PAD, n), np.float32)
    x[0:3] = posT
    for i in range(FREQ_NUM):
        f = np.float32(float(2 ** i) * np.pi)
        fx = f * posT
        x[3 + 6 * i: 6 + 6 * i] = np.sin(fx)
        x[6 + 6 * i: 9 + 6 * i] = np.cos(fx)
    return x


def _chunk_inputs(w0, b0, w1, b1, w2, b2, w3, b3, wf, bf):
    f32 = lambda a: np.ascontiguousarray(np.asarray(a), dtype=np.float32)
    w0p = np.zeros((D_PAD, HID), np.float32)
    w0p[:D_IN] = f32(w0)
    out = {"w0": w0p}
    for name, w in (("w1c", w1), ("w2c", w2), ("w3c", w3)):
        w = f32(w)
        out[name] = np.ascontiguousarray(np.stack([w[:128], w[128:]]))
    for name, b in (("b0c", b0), ("b1c", b1), ("b2c", b2), ("b3c", b3)):
        b = f32(b)
        out[name] = np.ascontiguousarray(b.reshape(2, 128).T)
    wf = f32(wf).reshape(-1)
    out["wfc"] = np.ascontiguousarray(wf.reshape(2, 128).T)
    out["bfc"] = f32(bf).reshape(1, 1)
    return out


def _sdf_device(x_t, weight_maps):
    """x_t: [27, N_VERTS] fp32 -> sdf [N_VERTS] fp32 via 8-core SPMD."""
    nc = _get_nc()
    in_maps = []
    for c in range(N_CORES):
        xc = np.zeros((D_PAD, PER_CORE_PAD), np.float32)
        xc[:, :PER_CORE] = x_t[:, c * PER_CORE:(c + 1) * PER_CORE]
        m = {"x": xc}
        m.update(weight_maps)
        in_maps.append(m)
    res = run_bass_kernel_spmd(nc, in_maps, list(range(N_CORES)))
    sdf = np.empty(N_VERTS, np.float32)
    for c in range(N_CORES):
        sdf[c * PER_CORE:(c + 1) * PER_CORE] = \
            res.results[c]["sdf"][0, :PER_CORE]
    return sdf


def _march_tets(pos, tet_fx4, sdf):
    """Marching tetrahedra, numpy mirror of the jnp reference."""
    occ = sdf > 0
    occ_fx4 = occ[tet_fx4]                       # [F,4]
    occ_sum = occ_fx4.sum(-1)
    valid = (occ_sum > 0) & (occ_sum < 4)
    vt = tet_fx4[valid]                          # [V,4]
    occ_v = occ_fx4[valid]

    e = vt[:, _BASE_EDGES].reshape(-1, 2)        # [6V,2]
    lo = np.minimum(e[:, 0], e[:, 1]).astype(np.int64)
    hi = np.maximum(e[:, 0], e[:, 1]).astype(np.int64)
    key = lo * N_VERTS + hi
    uniq_key, inv = np.unique(key, return_inverse=True)
    u_lo = (uniq_key // N_VERTS).astype(np.int64)
    u_hi = (uniq_key % N_VERTS).astype(np.int64)

    cross = (occ[u_lo].astype(np.int32) + occ[u_hi].astype(np.int32)) == 1
    mapping = np.where(cross, np.cumsum(cross, dtype=np.int64) - 1, -1)
    idx_map = mapping[inv].reshape(-1, 6).astype(np.int32)

    ilo = u_lo[cross]
    ihi = u_hi[cross]
    p0 = pos[ilo]
    p1 = pos[ihi]
    s0 = sdf[ilo]
    s1 = sdf[ihi]
    denom = s0 - s1
    verts = (p0 * (-s1)[:, None] + p1 * s0[:, None]) / denom[:, None]

    tetindex = (occ_v.astype(np.int32) *
                np.array([1, 2, 4, 8], np.int32)).sum(-1)
    ntri = _NUM_TRI[tetindex]
    tri = _TRI_TABLE[tetindex]
    m1 = ntri == 1
    m2 = ntri == 2
    f1 = np.take_along_axis(idx_map[m1], tri[m1][:, :3], axis=1).reshape(-1, 3)
    f2 = np.take_along_axis(idx_map[m2], tri[m2][:, :6], axis=1).reshape(-1, 3)
    faces = np.concatenate([f1, f2], axis=0).astype(np.int32)
    return verts.astype(np.float32), faces


def kernel(pos, tet_fx4, w0, b0, w1, b1, w2, b2, w3, b3, wf, bf):
    pos = np.ascontiguousarray(np.asarray(pos), dtype=np.float32)
    tet_fx4 = np.ascontiguousarray(np.asarray(tet_fx4), dtype=np.int32)
    x_t = _posenc_t(pos)
    wm = _chunk_inputs(w0, b0, w1, b1, w2, b2, w3, b3, wf, bf)
    sdf = _sdf_device(x_t, wm)
    return _march_tets(pos, tet_fx4, sdf)
